# revision 24
# baseline (speedup 1.0000x reference)
"""Causal multi-head attention (B=2, S=2048, D=1024, 16 heads of 64) on 8 TRN2
NeuronCores.

Sharding: core c -> batch b = c//4, head-group g = c%4 (4 heads = 256 model
dims per core).  Wq/Wk/Wv column-parallel, Wo row-parallel; the 4 partial
outputs per batch are summed on the host (no collectives).

Per-core data flow (matmul compute in bf16, fp32 PSUM accumulation):
  QT = (Wq_g/8) @ x^T      [256, 2048]   (1/sqrt(hd) folded into Wq,bq)
  KT = Wk_g @ x^T          [256, 2048]
  V  = x @ Wv_g^T + bv     [2048, 256]   natural layout, ones-augmented
  attention per head pair (64-row PE tiling throughout -> no PE mode
  switches, score pairs and AV halves run concurrently in the array):
    ST[sk,sq] = K_h @ Q_h^T          two heads at row groups 0/64
    P = exp(ST + causal mask)        one ACT exp per [128,1024] (2 banks)
    AV: split sk into two K=64 halves accumulating in separate PSUM banks
        (row-tile bank rule); lhsT is ones-augmented V so row 64 = l[sq]
    preoutT = (poA+poB)[0:64] * 1/(lA+lB)
  out_partial = preoutT.T @ Wo_g^T   [2048, 1024] fp32
Host: out[b] = sum of the 4 head-group partials + bo.
"""

import numpy as np
import ml_dtypes

B, S, D = 2, 2048, 1024
HD = 64
NH = D // HD
N_CORES = 8
GROUPS = 4          # head-groups (tensor-parallel)
JG = D // GROUPS    # local dims per core = 256
NHL = JG // HD      # local heads = 4
KCH = D // 128      # contraction chunks for projections = 8
NKT = S // 128      # sk tiles = 16
NJB = S // 512      # query blocks of 512 = 4
MASK_VAL = -1e9

BF16 = ml_dtypes.bfloat16

_cached = {}


def _build():
    import concourse.bacc as bacc
    import concourse.tile as tile
    import concourse.mybir as mybir

    f32 = mybir.dt.float32
    bf16 = mybir.dt.bfloat16
    Exp = mybir.ActivationFunctionType.Exp
    add_op = mybir.AluOpType.add

    nc = bacc.Bacc("TRN2", target_bir_lowering=False, debug=False,
                   num_devices=N_CORES)

    xT = nc.dram_tensor("xT", [128, KCH, S], bf16, kind="ExternalInput").ap()
    wqT = nc.dram_tensor("wqT", [128, KCH, JG], bf16, kind="ExternalInput").ap()
    wkT = nc.dram_tensor("wkT", [128, KCH, JG], bf16, kind="ExternalInput").ap()
    wvT = nc.dram_tensor("wvT", [128, KCH, JG], bf16, kind="ExternalInput").ap()
    woT = nc.dram_tensor("woT", [128, 2, D], bf16, kind="ExternalInput").ap()
    bqc = nc.dram_tensor("bqc", [JG, 1], f32, kind="ExternalInput").ap()
    bkc = nc.dram_tensor("bkc", [JG, 1], f32, kind="ExternalInput").ap()
    bvb = nc.dram_tensor("bvb", [128, JG], f32, kind="ExternalInput").ap()
    maskT = nc.dram_tensor("maskT", [128, 256], f32, kind="ExternalInput").ap()
    out = nc.dram_tensor("out", [S, D], f32, kind="ExternalOutput").ap()

    with tile.TileContext(nc) as tc:
        with (
            tc.tile_pool(name="const", bufs=1) as cpool,
            tc.tile_pool(name="pbig", bufs=3) as p_pool,
            tc.tile_pool(name="small", bufs=4) as small_pool,
            tc.tile_pool(name="outp", bufs=3) as out_pool,
            tc.tile_pool(name="mm_ps", bufs=2, space="PSUM") as mm_ps,
            tc.tile_pool(name="po_ps", bufs=4, space="PSUM") as po_ps,
        ):
            # ---- constants / weights in SBUF ----
            # DMA order matters: first QT-tile-0 deps (wq, bq, xt chunks in
            # k order), so the first projection matmul issues after ~1 MB of
            # DMA instead of after all inputs.
            wq_sb = cpool.tile([128, KCH, JG], bf16)
            nc.sync.dma_start(wq_sb[:], wqT[:])
            bq_sb = cpool.tile([128, 2], f32)
            nc.sync.dma_start(bq_sb[:], bqc.rearrange("(t p) o -> p (t o)", p=128))
            xt_all = cpool.tile([128, KCH, S], bf16)
            for k in range(KCH):
                nc.sync.dma_start(xt_all[:, k, :], xT[:, k, :])
            wk_sb = cpool.tile([128, KCH, JG], bf16)
            nc.sync.dma_start(wk_sb[:], wkT[:])
            bk_sb = cpool.tile([128, 2], f32)
            nc.sync.dma_start(bk_sb[:], bkc.rearrange("(t p) o -> p (t o)", p=128))
            wv_sb = cpool.tile([128, KCH, JG], bf16)
            nc.sync.dma_start(wv_sb[:], wvT[:])
            bvb_sb = cpool.tile([128, JG], f32)
            nc.sync.dma_start(bvb_sb[:], bvb[:])
            mask_sb = cpool.tile([128, 256], f32)
            nc.sync.dma_start(mask_sb[:], maskT[:])
            wo_sb = cpool.tile([128, 2, D], bf16)
            nc.sync.dma_start(wo_sb[:], woT[:])

            qt = [cpool.tile([128, S], bf16, name=f"qt{t}") for t in range(2)]
            kt = [cpool.tile([128, S], bf16, name=f"kt{t}") for t in range(2)]
            v_all = cpool.tile([128, NKT, NHL * 65], bf16)
            nc.vector.memset(
                v_all.rearrange("p k (h c) -> p k h c", c=65)[:, :, :, 64:65], 1.0)
            po = [cpool.tile([128, S], bf16, name=f"po{t}") for t in range(2)]

            # ---- phase helpers ----
            import concourse.bass as bass

            def proj_qkt(w_sb, b_sb, dst, t):
                ps = [mm_ps.tile([128, 1024], f32, tag="mm",
                                 name=f"psproj{t}_{n}") for n in range(2)]
                for k in range(KCH):
                    lhsT = w_sb[:, k, 128 * t:128 * t + 128]
                    for n in range(4):
                        nc.tensor.matmul(
                            ps[n // 2][:, 512 * (n % 2):512 * (n % 2) + 512],
                            lhsT=lhsT,
                            rhs=xt_all[:, k, 512 * n:512 * n + 512],
                            start=(k == 0), stop=(k == KCH - 1))
                for n in range(2):
                    nc.vector.tensor_scalar_add(
                        dst[t][:, 1024 * n:1024 * n + 1024], ps[n][:],
                        b_sb[:, t:t + 1])

            def proj_v(sg):
                # one accumulation group per PSUM bank: 2 s-tiles per tile,
                # parked at col 0 (bank 0) and col 512 (bank 1)
                ps = mm_ps.tile([128, 1024], f32, tag="mm", name=f"psv{sg}")
                for k in range(KCH):
                    for q in range(2):
                        si = 2 * sg + q
                        nc.tensor.matmul(
                            ps[:, 512 * q:512 * q + 256],
                            lhsT=xt_all[:, k, 128 * si:128 * si + 128],
                            rhs=wv_sb[:, k, :],
                            start=(k == 0), stop=(k == KCH - 1))
                for q in range(2):
                    si = 2 * sg + q
                    nc.vector.tensor_add(
                        v_all[:, si, :].rearrange(
                            "p (h c) -> p h c", c=65)[:, :, 0:64],
                        ps[:, 512 * q:512 * q + 256].rearrange(
                            "p (h c) -> p h c", c=64),
                        bvb_sb.rearrange("p (h c) -> p h c", c=64))

            def attn_block(pair, j):
                nk = 4 * (j + 1)
                npair = nk // 2
                qt_t, kt_t = qt[pair], kt[pair]
                pt = [p_pool.tile([128, NKT, 512], bf16, tag="p",
                                  name=f"pt{pair}_{j}_{hh}")
                      for hh in range(2)]
                # scores (K=64, head pair at row groups 0/64) + exp
                for m in range(npair):
                    sts = [mm_ps.tile([128, 1024], f32, tag="mm",
                                      name=f"st{pair}_{j}_{m}_{hh}")
                           for hh in range(2)]
                    for sub in range(2):
                        ki = 2 * m + sub
                        d = max(0, 128 * ki - 512 * j)
                        for hh in range(2):
                            base = 64 * hh
                            nc.tensor.matmul(
                                sts[hh][:, 512 * sub + d:512 * sub + 512],
                                lhsT=kt_t[base:base + 64,
                                          128 * ki:128 * ki + 128],
                                rhs=qt_t[base:base + 64,
                                         512 * j + d:512 * j + 512],
                                start=True, stop=True)
                    for hh in range(2):
                        if 2 * m >= 4 * j:
                            for sub in range(2):
                                ki = 2 * m + sub
                                d = 128 * ki - 512 * j
                                nc.vector.tensor_add(
                                    sts[hh][:, 512 * sub + d:512 * sub + d + 128],
                                    sts[hh][:, 512 * sub + d:512 * sub + d + 128],
                                    mask_sb[:, 0:128])
                        nc.scalar.activation(
                            pt[hh][:, 2 * m:2 * m + 2, :], sts[hh][:], Exp)
                # AV: K=128 contraction, one matmul per (ki, head);
                # po pool is double-buffered across blocks
                pos = [po_ps.tile([65, 512], f32, tag="po",
                                  name=f"pos{pair}_{j}_{hh}")
                       for hh in range(2)]
                for ki in range(nk):
                    d = max(0, 128 * ki - 512 * j)
                    for hh in range(2):
                        h = 2 * pair + hh
                        nc.tensor.matmul(
                            pos[hh][0:65, d:512],
                            lhsT=v_all[:, ki, 65 * h:65 * h + 65],
                            rhs=pt[hh][:, ki, d:512],
                            start=(ki == 0), stop=(ki == nk - 1))
                # evac: normalize by 1/l (row 64), write bf16
                for hh in range(2):
                    poh = pos[hh]
                    lrow = small_pool.tile([1, 512], f32, tag="lrow")
                    nc.vector.tensor_copy(lrow[:], poh[64:65, :])
                    rb = small_pool.tile([64, 512], f32, tag="rb")
                    nc.gpsimd.partition_broadcast(rb[:], lrow[:])
                    rbr = small_pool.tile([64, 512], f32, tag="rbr")
                    nc.vector.reciprocal_approx_fast(rbr[:], rb[:])
                    nc.vector.tensor_mul(
                        po[pair][64 * hh:64 * hh + 64,
                                 512 * j:512 * j + 512],
                        poh[0:64, :], rbr[:])

            def wo_block(wj):
                for m in range(4 * wj, 4 * wj + 4):
                    ps = mm_ps.tile([128, 1024], f32, tag="mm", name=f"pswo{m}")
                    for t in range(2):
                        for n in range(2):
                            nc.tensor.matmul(
                                ps[:, 512 * n:512 * n + 512],
                                lhsT=po[t][:, 128 * m:128 * m + 128],
                                rhs=wo_sb[:, t, 512 * n:512 * n + 512],
                                start=(t == 0), stop=(t == 1))
                    ob = out_pool.tile([128, 1024], f32, tag="ob")
                    if m % 2 == 0:
                        nc.vector.tensor_copy(ob[:], ps[:])
                    else:
                        nc.scalar.copy(ob[:], ps[:])
                    nc.sync.dma_start(out[128 * m:128 * m + 128, :], ob[:])

            # ---- schedule: projections interleaved with attention so the
            # scalar engine (exp) starts early and PE fills exp-paced gaps ----
            proj_qkt(wq_sb, bq_sb, qt, 0)
            proj_qkt(wk_sb, bk_sb, kt, 0)
            proj_v(0)
            proj_v(1)
            attn_block(0, 0)
            proj_qkt(wq_sb, bq_sb, qt, 1)
            proj_qkt(wk_sb, bk_sb, kt, 1)
            proj_v(2)
            proj_v(3)
            attn_block(1, 0)
            attn_block(0, 1)
            proj_v(4)
            proj_v(5)
            attn_block(1, 1)
            wo_block(0)
            proj_v(6)
            proj_v(7)
            attn_block(0, 2)
            attn_block(1, 2)
            wo_block(1)
            attn_block(0, 3)
            attn_block(1, 3)
            wo_block(2)
            wo_block(3)

    nc.compile()
    return nc


def _get_nc():
    if "nc" not in _cached:
        _cached["nc"] = _build()
    return _cached["nc"]


def _make_in_maps(x, Wq, bq, Wk, bk, Wv, bv, Wo):
    sc = 1.0 / np.sqrt(HD)
    tri = np.arange(128)
    mask1 = np.where(tri[:, None] <= tri[None, :], 0.0, MASK_VAL).astype(np.float32)
    mask = np.concatenate([mask1, mask1], axis=1)
    in_maps = []
    for c in range(N_CORES):
        b, g = divmod(c, GROUPS)
        sl = slice(JG * g, JG * (g + 1))
        def tile_k(a):  # [D, M] -> [128, D//128, M] contiguous
            return np.ascontiguousarray(
                a.reshape(a.shape[0] // 128, 128, a.shape[1]).transpose(1, 0, 2))

        in_maps.append({
            "xT": tile_k(x[b].T.astype(BF16)),
            "wqT": tile_k((Wq[sl] * sc).T.astype(BF16)),
            "wkT": tile_k(Wk[sl].T.astype(BF16)),
            "wvT": tile_k(Wv[sl].T.astype(BF16)),
            "woT": tile_k(Wo[:, sl].T.astype(BF16)),
            "bqc": (bq[sl] * sc).astype(np.float32).reshape(JG, 1),
            "bkc": bk[sl].astype(np.float32).reshape(JG, 1),
            "bvb": np.broadcast_to(bv[sl].astype(np.float32), (128, JG)).copy(),
            "maskT": mask,
        })
    return in_maps


def kernel(x, Wq, bq, Wk, bk, Wv, bv, Wo, bo, _return_results=False):
    from concourse.bass_utils import run_bass_kernel_spmd

    nc = _get_nc()
    in_maps = _make_in_maps(np.asarray(x, np.float32), np.asarray(Wq, np.float32),
                            np.asarray(bq, np.float32), np.asarray(Wk, np.float32),
                            np.asarray(bk, np.float32), np.asarray(Wv, np.float32),
                            np.asarray(bv, np.float32), np.asarray(Wo, np.float32))
    res = run_bass_kernel_spmd(nc, in_maps, core_ids=list(range(N_CORES)))
    full = np.empty((B, S, D), np.float32)
    for b in range(B):
        acc = res.results[4 * b]["out"].astype(np.float32).copy()
        for g in range(1, GROUPS):
            acc += res.results[4 * b + g]["out"]
        full[b] = acc + np.asarray(bo, np.float32)[None, :]
    if _return_results:
        return full, res
    return full


# revision 25
# speedup vs baseline: 1.0290x; 1.0290x over previous
"""Causal multi-head attention (B=2, S=2048, D=1024, 16 heads of 64) on 8 TRN2
NeuronCores.

Sharding: core c -> batch b = c//4, head-group g = c%4 (4 heads = 256 model
dims per core).  Wq/Wk/Wv column-parallel, Wo row-parallel; the 4 partial
outputs per batch are summed on the host (no collectives).

Per-core data flow (matmul compute in bf16, fp32 PSUM accumulation):
  QT = (Wq_g/8) @ x^T      [256, 2048]   (1/sqrt(hd) folded into Wq,bq)
  KT = Wk_g @ x^T          [256, 2048]
  V  = x @ Wv_g^T + bv     [2048, 256]   natural layout, ones-augmented
  attention per head pair (64-row PE tiling throughout -> no PE mode
  switches, score pairs and AV halves run concurrently in the array):
    ST[sk,sq] = K_h @ Q_h^T          two heads at row groups 0/64
    P = exp(ST + causal mask)        one ACT exp per [128,1024] (2 banks)
    AV: split sk into two K=64 halves accumulating in separate PSUM banks
        (row-tile bank rule); lhsT is ones-augmented V so row 64 = l[sq]
    preoutT = (poA+poB)[0:64] * 1/(lA+lB)
  out_partial = preoutT.T @ Wo_g^T   [2048, 1024] fp32
Host: out[b] = sum of the 4 head-group partials + bo.
"""

import numpy as np
import ml_dtypes

B, S, D = 2, 2048, 1024
HD = 64
NH = D // HD
N_CORES = 8
GROUPS = 4          # head-groups (tensor-parallel)
JG = D // GROUPS    # local dims per core = 256
NHL = JG // HD      # local heads = 4
KCH = D // 128      # contraction chunks for projections = 8
NKT = S // 128      # sk tiles = 16
NJB = S // 512      # query blocks of 512 = 4
MASK_VAL = -1e9

BF16 = ml_dtypes.bfloat16

_cached = {}


def _build():
    import concourse.bacc as bacc
    import concourse.tile as tile
    import concourse.mybir as mybir

    f32 = mybir.dt.float32
    bf16 = mybir.dt.bfloat16
    Exp = mybir.ActivationFunctionType.Exp
    add_op = mybir.AluOpType.add

    nc = bacc.Bacc("TRN2", target_bir_lowering=False, debug=False,
                   num_devices=N_CORES)

    xT = nc.dram_tensor("xT", [128, KCH, S], bf16, kind="ExternalInput").ap()
    wqT = nc.dram_tensor("wqT", [128, KCH, JG], bf16, kind="ExternalInput").ap()
    wkT = nc.dram_tensor("wkT", [128, KCH, JG], bf16, kind="ExternalInput").ap()
    wvT = nc.dram_tensor("wvT", [128, KCH, JG], bf16, kind="ExternalInput").ap()
    woT = nc.dram_tensor("woT", [128, 2, D], bf16, kind="ExternalInput").ap()
    bqc = nc.dram_tensor("bqc", [JG, 1], f32, kind="ExternalInput").ap()
    bkc = nc.dram_tensor("bkc", [JG, 1], f32, kind="ExternalInput").ap()
    bvb = nc.dram_tensor("bvb", [128, JG], f32, kind="ExternalInput").ap()
    maskT = nc.dram_tensor("maskT", [128, 256], f32, kind="ExternalInput").ap()
    out = nc.dram_tensor("out", [S, D], f32, kind="ExternalOutput").ap()

    with tile.TileContext(nc) as tc:
        with (
            tc.tile_pool(name="const", bufs=1) as cpool,
            tc.tile_pool(name="pbig", bufs=3) as p_pool,
            tc.tile_pool(name="small", bufs=4) as small_pool,
            tc.tile_pool(name="outp", bufs=3) as out_pool,
            tc.tile_pool(name="mm_ps", bufs=2, space="PSUM") as mm_ps,
            tc.tile_pool(name="po_ps", bufs=4, space="PSUM") as po_ps,
        ):
            # ---- constants / weights in SBUF ----
            # DMA order matters: first QT-tile-0 deps (wq, bq, xt chunks in
            # k order), so the first projection matmul issues after ~1 MB of
            # DMA instead of after all inputs.
            wq_sb = cpool.tile([128, KCH, JG], bf16)
            nc.sync.dma_start(wq_sb[:], wqT[:])
            bq_sb = cpool.tile([128, 2], f32)
            nc.sync.dma_start(bq_sb[:], bqc.rearrange("(t p) o -> p (t o)", p=128))
            xt_all = cpool.tile([128, KCH, S], bf16)
            for k in range(KCH):
                nc.sync.dma_start(xt_all[:, k, :], xT[:, k, :])
            wk_sb = cpool.tile([128, KCH, JG], bf16)
            nc.sync.dma_start(wk_sb[:], wkT[:])
            bk_sb = cpool.tile([128, 2], f32)
            nc.sync.dma_start(bk_sb[:], bkc.rearrange("(t p) o -> p (t o)", p=128))
            wv_sb = cpool.tile([128, KCH, JG], bf16)
            nc.sync.dma_start(wv_sb[:], wvT[:])
            bvb_sb = cpool.tile([128, JG], f32)
            nc.sync.dma_start(bvb_sb[:], bvb[:])
            mask_sb = cpool.tile([128, 256], f32)
            nc.sync.dma_start(mask_sb[:], maskT[:])
            wo_sb = cpool.tile([128, 2, D], bf16)
            nc.sync.dma_start(wo_sb[:], woT[:])

            qt = [cpool.tile([128, S], bf16, name=f"qt{t}") for t in range(2)]
            kt = [cpool.tile([128, S], bf16, name=f"kt{t}") for t in range(2)]
            v_all = cpool.tile([128, NKT, NHL * 65], bf16)
            nc.vector.memset(
                v_all.rearrange("p k (h c) -> p k h c", c=65)[:, :, :, 64:65], 1.0)
            po = [cpool.tile([128, S], bf16, name=f"po{t}") for t in range(2)]

            # ---- phase helpers ----
            import concourse.bass as bass

            def proj_qkt(w_sb, b_sb, dst, t):
                ps = [mm_ps.tile([128, 1024], f32, tag="mm",
                                 name=f"psproj{t}_{n}") for n in range(2)]
                for k in range(KCH):
                    lhsT = w_sb[:, k, 128 * t:128 * t + 128]
                    for n in range(4):
                        nc.tensor.matmul(
                            ps[n // 2][:, 512 * (n % 2):512 * (n % 2) + 512],
                            lhsT=lhsT,
                            rhs=xt_all[:, k, 512 * n:512 * n + 512],
                            start=(k == 0), stop=(k == KCH - 1))
                for n in range(2):
                    nc.vector.tensor_scalar_add(
                        dst[t][:, 1024 * n:1024 * n + 1024], ps[n][:],
                        b_sb[:, t:t + 1])

            def proj_v(sg):
                # one accumulation group per PSUM bank: 2 s-tiles per tile,
                # parked at col 0 (bank 0) and col 512 (bank 1)
                ps = mm_ps.tile([128, 1024], f32, tag="mm", name=f"psv{sg}")
                for k in range(KCH):
                    for q in range(2):
                        si = 2 * sg + q
                        nc.tensor.matmul(
                            ps[:, 512 * q:512 * q + 256],
                            lhsT=xt_all[:, k, 128 * si:128 * si + 128],
                            rhs=wv_sb[:, k, :],
                            start=(k == 0), stop=(k == KCH - 1))
                for q in range(2):
                    si = 2 * sg + q
                    nc.vector.tensor_add(
                        v_all[:, si, :].rearrange(
                            "p (h c) -> p h c", c=65)[:, :, 0:64],
                        ps[:, 512 * q:512 * q + 256].rearrange(
                            "p (h c) -> p h c", c=64),
                        bvb_sb.rearrange("p (h c) -> p h c", c=64))

            def attn_block(pair, j):
                nk = 4 * (j + 1)
                npair = nk // 2
                qt_t, kt_t = qt[pair], kt[pair]
                pt = [p_pool.tile([128, NKT, 512], bf16, tag="p",
                                  name=f"pt{pair}_{j}_{hh}")
                      for hh in range(2)]
                # scores (K=64, head pair at row groups 0/64) + exp
                for m in range(npair):
                    sts = [mm_ps.tile([128, 1024], f32, tag="mm",
                                      name=f"st{pair}_{j}_{m}_{hh}")
                           for hh in range(2)]
                    for sub in range(2):
                        ki = 2 * m + sub
                        d = max(0, 128 * ki - 512 * j)
                        for hh in range(2):
                            base = 64 * hh
                            nc.tensor.matmul(
                                sts[hh][:, 512 * sub + d:512 * sub + 512],
                                lhsT=kt_t[base:base + 64,
                                          128 * ki:128 * ki + 128],
                                rhs=qt_t[base:base + 64,
                                         512 * j + d:512 * j + 512],
                                start=True, stop=True)
                    for hh in range(2):
                        if 2 * m >= 4 * j:
                            for sub in range(2):
                                ki = 2 * m + sub
                                d = 128 * ki - 512 * j
                                nc.vector.tensor_add(
                                    sts[hh][:, 512 * sub + d:512 * sub + d + 128],
                                    sts[hh][:, 512 * sub + d:512 * sub + d + 128],
                                    mask_sb[:, 0:128])
                        nc.scalar.activation(
                            pt[hh][:, 2 * m:2 * m + 2, :], sts[hh][:], Exp)
                # AV: two K=64 halves -> separate PSUM banks (row-tile rule)
                pos = [po_ps.tile([65, 512], f32, tag="po",
                                  name=f"pos{pair}_{j}_{hh}_{half}")
                       for hh in range(2) for half in range(2)]
                for ki in range(nk):
                    d = max(0, 128 * ki - 512 * j)
                    for hh in range(2):
                        h = 2 * pair + hh
                        for half in range(2):
                            pb = 64 * half
                            nc.tensor.matmul(
                                pos[2 * hh + half][0:65, d:512],
                                lhsT=v_all[pb:pb + 64, ki,
                                           65 * h:65 * h + 65],
                                rhs=pt[hh][pb:pb + 64, ki, d:512],
                                start=(ki == 0), stop=(ki == nk - 1))
                # evac: sum halves, normalize by 1/l, write bf16
                for hh in range(2):
                    poA, poB = pos[2 * hh], pos[2 * hh + 1]
                    bsb = small_pool.tile([65, 512], f32, tag="bsb")
                    nc.vector.tensor_copy(bsb[:], poB[:])
                    ssum = small_pool.tile([65, 512], f32, tag="ssum")
                    nc.vector.tensor_add(ssum[:], poA[:], bsb[:])
                    lrow = small_pool.tile([1, 512], f32, tag="lrow")
                    nc.vector.tensor_copy(lrow[:], ssum[64:65, :])
                    rb = small_pool.tile([64, 512], f32, tag="rb")
                    nc.gpsimd.partition_broadcast(rb[:], lrow[:])
                    rbr = small_pool.tile([64, 512], f32, tag="rbr")
                    nc.vector.reciprocal_approx_fast(rbr[:], rb[:])
                    nc.vector.tensor_mul(
                        po[pair][64 * hh:64 * hh + 64,
                                 512 * j:512 * j + 512],
                        ssum[0:64, :], rbr[:])

            def wo_block(wj):
                for m in range(4 * wj, 4 * wj + 4):
                    ps = mm_ps.tile([128, 1024], f32, tag="mm", name=f"pswo{m}")
                    for t in range(2):
                        for n in range(2):
                            nc.tensor.matmul(
                                ps[:, 512 * n:512 * n + 512],
                                lhsT=po[t][:, 128 * m:128 * m + 128],
                                rhs=wo_sb[:, t, 512 * n:512 * n + 512],
                                start=(t == 0), stop=(t == 1))
                    ob = out_pool.tile([128, 1024], f32, tag="ob")
                    if m % 2 == 0:
                        nc.vector.tensor_copy(ob[:], ps[:])
                    else:
                        nc.scalar.copy(ob[:], ps[:])
                    nc.sync.dma_start(out[128 * m:128 * m + 128, :], ob[:])

            # ---- schedule: projections interleaved with attention so the
            # scalar engine (exp) starts early and PE fills exp-paced gaps ----
            proj_qkt(wq_sb, bq_sb, qt, 0)
            proj_qkt(wk_sb, bk_sb, kt, 0)
            proj_v(0)
            proj_v(1)
            proj_v(2)
            proj_v(3)
            attn_block(0, 1)
            proj_qkt(wq_sb, bq_sb, qt, 1)
            proj_qkt(wk_sb, bk_sb, kt, 1)
            attn_block(1, 1)
            proj_v(4)
            proj_v(5)
            attn_block(0, 2)
            proj_v(6)
            proj_v(7)
            attn_block(1, 2)
            wo_block(1)
            attn_block(0, 3)
            wo_block(2)
            attn_block(1, 3)
            attn_block(0, 0)
            wo_block(3)
            attn_block(1, 0)
            wo_block(0)

    nc.compile()
    return nc


def _get_nc():
    if "nc" not in _cached:
        _cached["nc"] = _build()
    return _cached["nc"]


def _make_in_maps(x, Wq, bq, Wk, bk, Wv, bv, Wo):
    sc = 1.0 / np.sqrt(HD)
    tri = np.arange(128)
    mask1 = np.where(tri[:, None] <= tri[None, :], 0.0, MASK_VAL).astype(np.float32)
    mask = np.concatenate([mask1, mask1], axis=1)
    in_maps = []
    for c in range(N_CORES):
        b, g = divmod(c, GROUPS)
        sl = slice(JG * g, JG * (g + 1))
        def tile_k(a):  # [D, M] -> [128, D//128, M] contiguous
            return np.ascontiguousarray(
                a.reshape(a.shape[0] // 128, 128, a.shape[1]).transpose(1, 0, 2))

        in_maps.append({
            "xT": tile_k(x[b].T.astype(BF16)),
            "wqT": tile_k((Wq[sl] * sc).T.astype(BF16)),
            "wkT": tile_k(Wk[sl].T.astype(BF16)),
            "wvT": tile_k(Wv[sl].T.astype(BF16)),
            "woT": tile_k(Wo[:, sl].T.astype(BF16)),
            "bqc": (bq[sl] * sc).astype(np.float32).reshape(JG, 1),
            "bkc": bk[sl].astype(np.float32).reshape(JG, 1),
            "bvb": np.broadcast_to(bv[sl].astype(np.float32), (128, JG)).copy(),
            "maskT": mask,
        })
    return in_maps


def kernel(x, Wq, bq, Wk, bk, Wv, bv, Wo, bo, _return_results=False):
    from concourse.bass_utils import run_bass_kernel_spmd

    nc = _get_nc()
    in_maps = _make_in_maps(np.asarray(x, np.float32), np.asarray(Wq, np.float32),
                            np.asarray(bq, np.float32), np.asarray(Wk, np.float32),
                            np.asarray(bk, np.float32), np.asarray(Wv, np.float32),
                            np.asarray(bv, np.float32), np.asarray(Wo, np.float32))
    res = run_bass_kernel_spmd(nc, in_maps, core_ids=list(range(N_CORES)))
    full = np.empty((B, S, D), np.float32)
    for b in range(B):
        acc = res.results[4 * b]["out"].astype(np.float32).copy()
        for g in range(1, GROUPS):
            acc += res.results[4 * b + g]["out"]
        full[b] = acc + np.asarray(bo, np.float32)[None, :]
    if _return_results:
        return full, res
    return full


# revision 26
# speedup vs baseline: 1.0978x; 1.0668x over previous
"""Causal multi-head attention (B=2, S=2048, D=1024, 16 heads of 64) on 8 TRN2
NeuronCores.

Sharding: core c -> batch b = c//4, head-group g = c%4 (4 heads = 256 model
dims per core).  Wq/Wk/Wv column-parallel, Wo row-parallel; the 4 partial
outputs per batch are summed on the host (no collectives).

Per-core data flow (matmul compute in bf16, fp32 PSUM accumulation):
  QT = (Wq_g/8) @ x^T      [256, 2048]   (1/sqrt(hd) folded into Wq,bq)
  KT = Wk_g @ x^T          [256, 2048]
  V  = x @ Wv_g^T + bv     [2048, 256]   natural layout, ones-augmented
  attention per head pair (64-row PE tiling throughout -> no PE mode
  switches, score pairs and AV halves run concurrently in the array):
    ST[sk,sq] = K_h @ Q_h^T          two heads at row groups 0/64
    P = exp(ST + causal mask)        one ACT exp per [128,1024] (2 banks)
    AV: split sk into two K=64 halves accumulating in separate PSUM banks
        (row-tile bank rule); lhsT is ones-augmented V so row 64 = l[sq]
    preoutT = (poA+poB)[0:64] * 1/(lA+lB)
  out_partial = preoutT.T @ Wo_g^T   [2048, 1024] fp32
Host: out[b] = sum of the 4 head-group partials + bo.
"""

import numpy as np
import ml_dtypes

B, S, D = 2, 2048, 1024
HD = 64
NH = D // HD
N_CORES = 8
GROUPS = 4          # head-groups (tensor-parallel)
JG = D // GROUPS    # local dims per core = 256
NHL = JG // HD      # local heads = 4
KCH = D // 128      # contraction chunks for projections = 8
NKT = S // 128      # sk tiles = 16
NJB = S // 512      # query blocks of 512 = 4
MASK_VAL = -1e9

BF16 = ml_dtypes.bfloat16

_cached = {}


def _build():
    import concourse.bacc as bacc
    import concourse.tile as tile
    import concourse.mybir as mybir

    f32 = mybir.dt.float32
    bf16 = mybir.dt.bfloat16
    Exp = mybir.ActivationFunctionType.Exp
    add_op = mybir.AluOpType.add

    nc = bacc.Bacc("TRN2", target_bir_lowering=False, debug=False,
                   num_devices=N_CORES)

    xT = nc.dram_tensor("xT", [128, KCH, S], bf16, kind="ExternalInput").ap()
    wqT = nc.dram_tensor("wqT", [128, KCH, JG], bf16, kind="ExternalInput").ap()
    wkT = nc.dram_tensor("wkT", [128, KCH, JG], bf16, kind="ExternalInput").ap()
    wvT = nc.dram_tensor("wvT", [128, KCH, JG], bf16, kind="ExternalInput").ap()
    woT = nc.dram_tensor("woT", [128, 2, D], bf16, kind="ExternalInput").ap()
    bqc = nc.dram_tensor("bqc", [JG, 1], f32, kind="ExternalInput").ap()
    bkc = nc.dram_tensor("bkc", [JG, 1], f32, kind="ExternalInput").ap()
    bvb = nc.dram_tensor("bvb", [128, JG], f32, kind="ExternalInput").ap()
    maskT = nc.dram_tensor("maskT", [128, 256], f32, kind="ExternalInput").ap()
    out = nc.dram_tensor("out", [S, D], f32, kind="ExternalOutput").ap()

    with tile.TileContext(nc) as tc:
        with (
            tc.tile_pool(name="const", bufs=1) as cpool,
            tc.tile_pool(name="pbig", bufs=4) as p_pool,
            tc.tile_pool(name="small", bufs=4) as small_pool,
            tc.tile_pool(name="outp", bufs=3) as out_pool,
            tc.tile_pool(name="mm_ps", bufs=2, space="PSUM") as mm_ps,
            tc.tile_pool(name="po_ps", bufs=4, space="PSUM") as po_ps,
        ):
            # ---- constants / weights in SBUF ----
            # DMA order matters: first QT-tile-0 deps (wq, bq, xt chunks in
            # k order), so the first projection matmul issues after ~1 MB of
            # DMA instead of after all inputs.
            wq_sb = cpool.tile([128, KCH, JG], bf16)
            nc.sync.dma_start(wq_sb[:], wqT[:])
            bq_sb = cpool.tile([128, 2], f32)
            nc.sync.dma_start(bq_sb[:], bqc.rearrange("(t p) o -> p (t o)", p=128))
            xt_all = cpool.tile([128, KCH, S], bf16)
            for k in range(KCH):
                nc.sync.dma_start(xt_all[:, k, :], xT[:, k, :])
            wk_sb = cpool.tile([128, KCH, JG], bf16)
            nc.sync.dma_start(wk_sb[:], wkT[:])
            bk_sb = cpool.tile([128, 2], f32)
            nc.sync.dma_start(bk_sb[:], bkc.rearrange("(t p) o -> p (t o)", p=128))
            wv_sb = cpool.tile([128, KCH, JG], bf16)
            nc.sync.dma_start(wv_sb[:], wvT[:])
            bvb_sb = cpool.tile([128, JG], f32)
            nc.sync.dma_start(bvb_sb[:], bvb[:])
            mask_sb = cpool.tile([128, 256], f32)
            nc.sync.dma_start(mask_sb[:], maskT[:])
            wo_sb = cpool.tile([128, 2, D], bf16)
            nc.sync.dma_start(wo_sb[:], woT[:])

            qt = [cpool.tile([128, S], bf16, name=f"qt{t}") for t in range(2)]
            kt = [cpool.tile([128, S], bf16, name=f"kt{t}") for t in range(2)]
            v_all = cpool.tile([128, NKT, NHL * 65], bf16)
            nc.vector.memset(
                v_all.rearrange("p k (h c) -> p k h c", c=65)[:, :, :, 64:65], 1.0)
            po = [cpool.tile([128, S], bf16, name=f"po{t}") for t in range(2)]

            # ---- phase helpers ----
            import concourse.bass as bass

            def proj_qkt(w_sb, b_sb, dst, t):
                ps = [mm_ps.tile([128, 1024], f32, tag="mm",
                                 name=f"psproj{t}_{n}") for n in range(2)]
                for k in range(KCH):
                    lhsT = w_sb[:, k, 128 * t:128 * t + 128]
                    for n in range(4):
                        nc.tensor.matmul(
                            ps[n // 2][:, 512 * (n % 2):512 * (n % 2) + 512],
                            lhsT=lhsT,
                            rhs=xt_all[:, k, 512 * n:512 * n + 512],
                            start=(k == 0), stop=(k == KCH - 1))
                for n in range(2):
                    nc.vector.tensor_scalar_add(
                        dst[t][:, 1024 * n:1024 * n + 1024], ps[n][:],
                        b_sb[:, t:t + 1])

            def proj_v(sg):
                # one accumulation group per PSUM bank: 2 s-tiles per tile,
                # parked at col 0 (bank 0) and col 512 (bank 1)
                ps = mm_ps.tile([128, 1024], f32, tag="mm", name=f"psv{sg}")
                for k in range(KCH):
                    for q in range(2):
                        si = 2 * sg + q
                        nc.tensor.matmul(
                            ps[:, 512 * q:512 * q + 256],
                            lhsT=xt_all[:, k, 128 * si:128 * si + 128],
                            rhs=wv_sb[:, k, :],
                            start=(k == 0), stop=(k == KCH - 1))
                for q in range(2):
                    si = 2 * sg + q
                    nc.vector.tensor_add(
                        v_all[:, si, :].rearrange(
                            "p (h c) -> p h c", c=65)[:, :, 0:64],
                        ps[:, 512 * q:512 * q + 256].rearrange(
                            "p (h c) -> p h c", c=64),
                        bvb_sb.rearrange("p (h c) -> p h c", c=64))

            def attn_block(pair, j):
                nk = 4 * (j + 1)
                npair = nk // 2
                qt_t, kt_t = qt[pair], kt[pair]
                pt = [p_pool.tile([128, NKT, 512], bf16, tag="p",
                                  name=f"pt{pair}_{j}_{hh}")
                      for hh in range(2)]
                # scores (K=64, head pair at row groups 0/64) + exp
                for m in range(npair):
                    sts = [mm_ps.tile([128, 1024], f32, tag="mm",
                                      name=f"st{pair}_{j}_{m}_{hh}")
                           for hh in range(2)]
                    for sub in range(2):
                        ki = 2 * m + sub
                        d = max(0, 128 * ki - 512 * j)
                        for hh in range(2):
                            base = 64 * hh
                            nc.tensor.matmul(
                                sts[hh][:, 512 * sub + d:512 * sub + 512],
                                lhsT=kt_t[base:base + 64,
                                          128 * ki:128 * ki + 128],
                                rhs=qt_t[base:base + 64,
                                         512 * j + d:512 * j + 512],
                                start=True, stop=True)
                    for hh in range(2):
                        if 2 * m >= 4 * j:
                            for sub in range(2):
                                ki = 2 * m + sub
                                d = 128 * ki - 512 * j
                                nc.vector.tensor_add(
                                    sts[hh][:, 512 * sub + d:512 * sub + d + 128],
                                    sts[hh][:, 512 * sub + d:512 * sub + d + 128],
                                    mask_sb[:, 0:128])
                        nc.scalar.activation(
                            pt[hh][:, 2 * m:2 * m + 2, :], sts[hh][:], Exp)
                # AV: two K=64 halves -> separate PSUM banks (row-tile rule)
                pos = [po_ps.tile([65, 512], f32, tag="po",
                                  name=f"pos{pair}_{j}_{hh}_{half}")
                       for hh in range(2) for half in range(2)]
                for ki in range(nk):
                    d = max(0, 128 * ki - 512 * j)
                    for hh in range(2):
                        h = 2 * pair + hh
                        for half in range(2):
                            pb = 64 * half
                            nc.tensor.matmul(
                                pos[2 * hh + half][0:65, d:512],
                                lhsT=v_all[pb:pb + 64, ki,
                                           65 * h:65 * h + 65],
                                rhs=pt[hh][pb:pb + 64, ki, d:512],
                                start=(ki == 0), stop=(ki == nk - 1))
                # evac: sum halves, normalize by 1/l, write bf16
                for hh in range(2):
                    poA, poB = pos[2 * hh], pos[2 * hh + 1]
                    bsb = small_pool.tile([65, 512], f32, tag="bsb")
                    nc.vector.tensor_copy(bsb[:], poB[:])
                    ssum = small_pool.tile([65, 512], f32, tag="ssum")
                    nc.vector.tensor_add(ssum[:], poA[:], bsb[:])
                    lrow = small_pool.tile([1, 512], f32, tag="lrow")
                    nc.vector.tensor_copy(lrow[:], ssum[64:65, :])
                    rb = small_pool.tile([64, 512], f32, tag="rb")
                    nc.gpsimd.partition_broadcast(rb[:], lrow[:])
                    rbr = small_pool.tile([64, 512], f32, tag="rbr")
                    nc.vector.reciprocal_approx_fast(rbr[:], rb[:])
                    nc.vector.tensor_mul(
                        po[pair][64 * hh:64 * hh + 64,
                                 512 * j:512 * j + 512],
                        ssum[0:64, :], rbr[:])

            def wo_block(wj):
                for m in range(4 * wj, 4 * wj + 4):
                    ps = mm_ps.tile([128, 1024], f32, tag="mm", name=f"pswo{m}")
                    for t in range(2):
                        for n in range(2):
                            nc.tensor.matmul(
                                ps[:, 512 * n:512 * n + 512],
                                lhsT=po[t][:, 128 * m:128 * m + 128],
                                rhs=wo_sb[:, t, 512 * n:512 * n + 512],
                                start=(t == 0), stop=(t == 1))
                    ob = out_pool.tile([128, 1024], f32, tag="ob")
                    if m % 2 == 0:
                        nc.vector.tensor_copy(ob[:], ps[:])
                    else:
                        nc.scalar.copy(ob[:], ps[:])
                    nc.sync.dma_start(out[128 * m:128 * m + 128, :], ob[:])

            # ---- schedule: projections interleaved with attention so the
            # scalar engine (exp) starts early and PE fills exp-paced gaps ----
            proj_qkt(wq_sb, bq_sb, qt, 0)
            proj_qkt(wk_sb, bk_sb, kt, 0)
            proj_v(0)
            proj_v(1)
            attn_block(0, 0)
            proj_qkt(wq_sb, bq_sb, qt, 1)
            proj_qkt(wk_sb, bk_sb, kt, 1)
            proj_v(2)
            proj_v(3)
            attn_block(1, 0)
            attn_block(0, 1)
            proj_v(4)
            proj_v(5)
            attn_block(1, 1)
            wo_block(0)
            proj_v(6)
            proj_v(7)
            attn_block(0, 2)
            attn_block(1, 2)
            wo_block(1)
            attn_block(0, 3)
            attn_block(1, 3)
            wo_block(2)
            wo_block(3)

    nc.compile()
    return nc


def _get_nc():
    if "nc" not in _cached:
        _cached["nc"] = _build()
    return _cached["nc"]


def _make_in_maps(x, Wq, bq, Wk, bk, Wv, bv, Wo):
    sc = 1.0 / np.sqrt(HD)
    tri = np.arange(128)
    mask1 = np.where(tri[:, None] <= tri[None, :], 0.0, MASK_VAL).astype(np.float32)
    mask = np.concatenate([mask1, mask1], axis=1)
    in_maps = []
    for c in range(N_CORES):
        b, g = divmod(c, GROUPS)
        sl = slice(JG * g, JG * (g + 1))
        def tile_k(a):  # [D, M] -> [128, D//128, M] contiguous
            return np.ascontiguousarray(
                a.reshape(a.shape[0] // 128, 128, a.shape[1]).transpose(1, 0, 2))

        in_maps.append({
            "xT": tile_k(x[b].T.astype(BF16)),
            "wqT": tile_k((Wq[sl] * sc).T.astype(BF16)),
            "wkT": tile_k(Wk[sl].T.astype(BF16)),
            "wvT": tile_k(Wv[sl].T.astype(BF16)),
            "woT": tile_k(Wo[:, sl].T.astype(BF16)),
            "bqc": (bq[sl] * sc).astype(np.float32).reshape(JG, 1),
            "bkc": bk[sl].astype(np.float32).reshape(JG, 1),
            "bvb": np.broadcast_to(bv[sl].astype(np.float32), (128, JG)).copy(),
            "maskT": mask,
        })
    return in_maps


def kernel(x, Wq, bq, Wk, bk, Wv, bv, Wo, bo, _return_results=False):
    from concourse.bass_utils import run_bass_kernel_spmd

    nc = _get_nc()
    in_maps = _make_in_maps(np.asarray(x, np.float32), np.asarray(Wq, np.float32),
                            np.asarray(bq, np.float32), np.asarray(Wk, np.float32),
                            np.asarray(bk, np.float32), np.asarray(Wv, np.float32),
                            np.asarray(bv, np.float32), np.asarray(Wo, np.float32))
    res = run_bass_kernel_spmd(nc, in_maps, core_ids=list(range(N_CORES)))
    full = np.empty((B, S, D), np.float32)
    for b in range(B):
        acc = res.results[4 * b]["out"].astype(np.float32).copy()
        for g in range(1, GROUPS):
            acc += res.results[4 * b + g]["out"]
        full[b] = acc + np.asarray(bo, np.float32)[None, :]
    if _return_results:
        return full, res
    return full
